# revision 17
# baseline (speedup 1.0000x reference)
"""Discrete Hawkes conditional-intensity kernel for 8 Trainium2 NeuronCores.

Math
----
Reference computes, per query i with (t, s) = (t_i, s_i):

    lam_i = clip(mu[s] + alpha[s, s] * b * F[t, s], 1e-5)
    F[t, s] = sum_{tp < t} obs[tp, s] * exp(-b * (t - tp))

With t = j*128 + p (j time-block of 128):

    F[j*128+p, s] = sum_{q<p} obs[j*128+q, s] e^{-b(p-q)}   (within block, PE)
                  + e^{-b p} * C[j, s]                       (carry)
    C[j, s] = F[j*128, s] = sum_{j'<j} e^{-128 b (j-1-j')} r[j', s]
    r[j, s] = sum_q obs[j*128+q, s] e^{-b(128-q)}

Sharding: by SPACE.  Core c owns s in [32c, 32c+32) — it reads only its
32 obs columns (1/8 of obs), builds its G table with one blocked matmul
pass (r and the carry C are two more small matmuls, not a sequential
chain), and stores it to DRAM as [4096, 64] f32 rows (32 used + 32 pad
so each row is one 256B gather element).  Queries (~8192/core, those
with s in the core's range) are routed by the host into per-s slot
groups of 384; one SWDGE dma_gather pulls each query's G row (t) into
rows[p, c, :] at DMA-engine rate, and a single strided-diagonal
tensor_scalar picks element s out of every row (the per-s grouping
makes the select offset affine in the slot index).  No collectives.
"""

import os
import sys

import numpy as np

_REPO_CANDIDATES = ("/opt/trn_rl_repo", os.path.expanduser("~/.axon_site/_ro/trn_rl_repo"))
for _p in _REPO_CANDIDATES:
    if os.path.isdir(_p) and _p not in sys.path:
        sys.path.append(_p)

import concourse.bass as bass
import concourse.tile as tile
from concourse import bacc, mybir
from concourse.bass_utils import run_bass_kernel_spmd

# Problem constants (hardcoded per spec).
N_TIME = 4096
N_SPACE = 256
BATCH = 65536
N_CORES = 8
LAM_MIN = 1e-5

P = 128                 # partitions / time-block size
J = N_TIME // P         # 32 time blocks
S = N_SPACE // N_CORES  # 32 space columns per core
W = 2 * S               # padded G row width (64 f32 = one 256B gather elem)
NC1 = 3                 # slot columns per s-group (384 slots; max seen 296)
NC = S * NC1            # 96 slot columns
NSLOT = P * NC          # 12288 gather slots per core

f32 = mybir.dt.float32
bf16 = mybir.dt.bfloat16
i32 = mybir.dt.int32
i16 = mybir.dt.int16
i8 = mybir.dt.int8
Alu = mybir.AluOpType
Act = mybir.ActivationFunctionType


def build_nc():
    nc = bacc.Bacc("TRN2", target_bir_lowering=False, debug=False)

    idx_h = nc.dram_tensor("idx", [P, NSLOT // 16], i16, kind="ExternalInput")
    obs1_h = nc.dram_tensor("obs1", [P, J * S], i8, kind="ExternalInput")
    par_h = nc.dram_tensor("par", [2, S], f32, kind="ExternalInput")  # mu; adiag
    beta_h = nc.dram_tensor("beta", [1], f32, kind="ExternalInput")
    g_h = nc.dram_tensor("gtab", [N_TIME * W], f32, kind="Internal")
    out_h = nc.dram_tensor("out", [P * NC], f32, kind="ExternalOutput")

    from contextlib import ExitStack

    with tile.TileContext(nc) as tc, ExitStack() as ctx:
        sb = ctx.enter_context(tc.tile_pool(name="sb", bufs=1))
        ps = ctx.enter_context(tc.tile_pool(name="ps", bufs=2, space="PSUM"))
        ps1 = ctx.enter_context(tc.tile_pool(name="ps1", bufs=2, space="PSUM"))

        # ---- input loads ------------------------------------------------
        beta_bc = sb.tile([P, 1], f32)
        nc.sync.dma_start(beta_bc[:], bass.AP(beta_h, 0, [[0, P], [1, 1]]))
        par = sb.tile([1, 2 * S], f32)   # [mu | adiag] on one partition
        nc.sync.dma_start(par[:], bass.AP(par_h, 0, [[1, 1], [1, 2 * S]]))
        obs1_i = sb.tile([P, J * S], i8)
        nc.sync.dma_start(obs1_i[:], obs1_h.ap())
        idx = sb.tile([P, NSLOT // 16], i16)
        nc.scalar.dma_start(idx[:], idx_h.ap())

        # ---- runtime constants from beta --------------------------------
        negb = sb.tile([P, 1], f32)
        nc.vector.tensor_scalar(out=negb[:], in0=beta_bc[:], scalar1=-1.0,
                                scalar2=None, op0=Alu.mult)
        negb128 = sb.tile([J, 1], f32)
        nc.vector.tensor_scalar(out=negb128[:], in0=beta_bc[:J, :], scalar1=-128.0,
                                scalar2=None, op0=Alu.mult)

        # broadcast adiag to all 128 partitions via PE; asbb[s] = b*alpha[s,s]
        ones1 = sb.tile([1, P], f32)
        nc.vector.memset(ones1[:], 1.0)
        bc_ps = ps1.tile([P, S], f32)
        nc.tensor.matmul(bc_ps[:], lhsT=ones1[:], rhs=par[:, S:2 * S],
                         start=True, stop=True)
        asbb_bc = sb.tile([P, S], f32)
        nc.vector.tensor_scalar(out=asbb_bc[:], in0=bc_ps[:],
                                scalar1=beta_bc[:], scalar2=None, op0=Alu.mult)

        # v column: exp(b*(p-128))  (end-of-block carry weights)
        xvc = sb.tile([P, 1], i32)
        nc.gpsimd.iota(xvc[:], [[0, 1]], base=-P, channel_multiplier=1)
        vmc = sb.tile([P, 1], f32)
        nc.vector.tensor_scalar(out=vmc[:], in0=xvc[:], scalar1=beta_bc[:],
                                scalar2=None, op0=Alu.mult)
        vcolb = sb.tile([P, 1], bf16)
        nc.scalar.activation(vcolb[:], vmc[:], Act.Exp)

        # u2b: row0 = exp(-b p) (carry decay), row1 = ones (mu term)
        xu = sb.tile([1, P], i32)
        nc.gpsimd.iota(xu[:], [[1, P]], base=0, channel_multiplier=0)
        u2b = sb.tile([2, P], bf16)
        nc.vector.memset(u2b[:], 1.0)
        um = sb.tile([1, P], f32)
        nc.vector.tensor_scalar(out=um[:], in0=xu[:], scalar1=negb[:1, :],
                                scalar2=None, op0=Alu.mult)
        nc.scalar.activation(u2b[0:1, :], um[:], Act.Exp)

        # LdT[q, p] = exp(-b (p - q)) for q < p else 0   (within-block decay)
        xd = sb.tile([P, P], i32)
        nc.gpsimd.iota(xd[:], [[1, P]], base=0, channel_multiplier=-1)   # f - p
        lda = sb.tile([P, P], f32)
        nc.vector.tensor_scalar(out=lda[:], in0=xd[:], scalar1=negb[:],
                                scalar2=None, op0=Alu.mult)
        ldm = sb.tile([P, P], f32)
        nc.gpsimd.affine_select(ldm[:], lda[:], [[1, P]], Alu.is_gt, -90.0,
                                base=0, channel_multiplier=-1)
        ldtb = sb.tile([P, P], bf16)
        nc.scalar.activation(ldtb[:], ldm[:], Act.Exp)

        # K[j', j] = exp(-128 b (j - 1 - j')) for j' <= j-1 else 0  (carry)
        xc = sb.tile([J, J], i32)
        nc.gpsimd.iota(xc[:], [[1, J]], base=-1, channel_multiplier=-1)  # f - 1 - p
        lca = sb.tile([J, J], f32)
        nc.vector.tensor_scalar(out=lca[:], in0=xc[:], scalar1=negb128[:],
                                scalar2=None, op0=Alu.mult)
        lcm = sb.tile([J, J], f32)
        nc.gpsimd.affine_select(lcm[:], lca[:], [[1, J]], Alu.is_ge, -90.0,
                                base=-1, channel_multiplier=-1)
        kct = sb.tile([J, J], f32)
        nc.scalar.activation(kct[:], lcm[:], Act.Exp)

        # ---- obs scale + carry path (all matmuls) -----------------------
        # obs_f1[p, (j, s)] = obs1 * asbb[s]
        obs_f1 = sb.tile([P, J * S], bf16)
        nc.vector.tensor_tensor(
            out=obs_f1[:].rearrange("p (j s) -> p j s", s=S),
            in0=obs1_i[:].rearrange("p (j s) -> p j s", s=S),
            in1=asbb_bc[:].unsqueeze(1).broadcast_to((P, J, S)),
            op=Alu.mult)

        HALF = J * S // 2   # 512 free elems per PSUM bank
        # r[(j, s)] = sum_q obs_f1[q, (j, s)] * v[q]
        r_flat = sb.tile([1, J * S], f32)
        for h in range(2):
            r_ps = ps.tile([1, HALF], f32)
            nc.tensor.matmul(r_ps[:], lhsT=vcolb[:],
                             rhs=obs_f1[:, h * HALF:(h + 1) * HALF],
                             start=True, stop=True)
            if h == 0:
                nc.scalar.activation(r_flat[:, 0:HALF], r_ps[:], Act.Copy)
            else:
                nc.vector.tensor_copy(r_flat[:, HALF:2 * HALF], r_ps[:])
        r32 = sb.tile([J, S], f32)
        nc.sync.dma_start(r32[:], r_flat[:])
        c_ps = ps1.tile([J, S], f32)
        nc.tensor.matmul(c_ps[:], lhsT=kct[:], rhs=r32[:], start=True, stop=True)
        c_sb = sb.tile([J, S], bf16)
        nc.vector.tensor_copy(c_sb[:], c_ps[:])

        # rhs2: row0 = C flat (sbuf reshape dma), row1 = mu tiled (dma bcast)
        rhs2 = sb.tile([2, J * S], bf16)
        mu_b = sb.tile([1, S], bf16)
        nc.vector.tensor_copy(mu_b[:], par[:, 0:S])
        nc.scalar.dma_start(
            rhs2[1:2, :].rearrange("o (j s) -> o j s", s=S),
            mu_b[:].unsqueeze(1).broadcast_to((1, J, S)))
        nc.sync.dma_start(
            rhs2[0:1, :].rearrange("o (j s) -> o j s", s=S), c_sb[:])

        # ---- G build + store to DRAM ------------------------------------
        g_store = bass.AP(g_h, 0, [[W, P], [P * W, J], [1, S]])  # [p, j, s<32]
        g_sb = sb.tile([P, J * S], f32)
        for h in range(2):
            pch = ps.tile([P, HALF], f32)
            nc.tensor.matmul(pch[:], lhsT=ldtb[:],
                             rhs=obs_f1[:, h * HALF:(h + 1) * HALF],
                             start=True, stop=True)
            nc.tensor.matmul(pch[:], lhsT=u2b[:],
                             rhs=rhs2[:, h * HALF:(h + 1) * HALF],
                             start=False, stop=True, skip_group_check=True)
            if h == 0:
                nc.vector.tensor_copy(g_sb[:, 0:HALF], pch[:])
                eng = nc.sync
            else:
                nc.scalar.activation(g_sb[:, HALF:2 * HALF], pch[:], Act.Copy)
                eng = nc.scalar
            eng.dma_start(
                g_store[:, h * (J // 2):(h + 1) * (J // 2), :],
                g_sb[:, h * HALF:(h + 1) * HALF].rearrange(
                    "p (j s) -> p j s", s=S))

        # ---- row gather + diagonal extract + out ------------------------
        rows = sb.tile([P, NC * W], f32)
        nc.gpsimd.dma_gather(
            out_ap=rows[:].rearrange("p (c e) -> p c e", e=W),
            in_ap=bass.AP(g_h, 0, [[W, N_TIME], [1, W]]),
            idxs_ap=idx[:],
            num_idxs=NSLOT,
            num_idxs_reg=NSLOT,
            elem_size=W,
            single_packet=False,
        )
        # slot layout groups queries by s: s-group sg occupies columns
        # [NC1*sg, NC1*(sg+1)), and every row in those columns needs element
        # sg — an affine (diagonal) read: offset = sg*(NC1*W+1) + c*W.
        lam = sb.tile([P, NC], f32)
        diag = bass.AP(rows[:].tensor, 0,
                       [[NC * W, P], [NC1 * W + 1, S], [W, NC1]])
        nc.vector.tensor_scalar(
            out=lam[:].rearrange("p (a b) -> p a b", b=NC1),
            in0=diag, scalar1=float(LAM_MIN), scalar2=None, op0=Alu.max)
        nc.sync.dma_start(bass.AP(out_h, 0, [[NC, P], [1, NC]]), lam[:])

    nc.compile()
    return nc


_NC_CACHE = None


def _get_nc():
    global _NC_CACHE
    if _NC_CACHE is None:
        _NC_CACHE = build_nc()
    return _NC_CACHE


def _route_queries(tc_, sc_):
    """Route a core's queries to dma_gather slots, grouped by s.

    Slot i gets row t_i gathered into rows[i % 128, i // 128, :]; queries
    with s_rel = sg occupy slots [384*sg, 384*sg + count_sg) so the final
    extract offset is affine in the slot.  Returns (idx_dev [P, NSLOT//16]
    int16 (wrapped in 16 partitions, replicated to all 8 groups), flat out
    positions p*NC + col per query in input order)."""
    n = tc_.shape[0]
    counts = np.bincount(sc_, minlength=S)
    if counts.max() > NC1 * P:
        raise RuntimeError("per-s query count exceeds slot capacity")
    order = np.argsort(sc_, kind="stable")
    starts = np.zeros(S, np.int64)
    np.cumsum(counts[:-1], out=starts[1:])
    rank = np.empty(n, np.int64)
    rank[order] = np.arange(n) - starts[sc_[order]]
    slot = sc_.astype(np.int64) * (NC1 * P) + rank
    idx_list = np.zeros(NSLOT, np.int16)
    idx_list[slot] = tc_.astype(np.int16)
    wrapped = np.ascontiguousarray(idx_list.reshape(NSLOT // 16, 16).T)
    idx_dev = np.tile(wrapped, (P // 16, 1))
    flat_pos = (slot % P) * NC + (slot >> 7)
    return idx_dev, flat_pos


def _make_in_maps(t, s, obs, mu, alpha, beta):
    """Shard by space: core c gets s in [S*c, S*(c+1)).  Returns
    (in_maps, perms) where perms[c] = (flat_out_pos, global_orig_pos)."""
    t = np.ascontiguousarray(np.asarray(t, dtype=np.int32))
    s = np.ascontiguousarray(np.asarray(s, dtype=np.int32))
    obs = np.ascontiguousarray(np.asarray(obs, dtype=np.int32))
    mu = np.ascontiguousarray(np.asarray(mu, dtype=np.float32))
    alpha = np.asarray(alpha, dtype=np.float32)
    beta = np.ascontiguousarray(np.asarray(beta, dtype=np.float32))
    adiag = np.ascontiguousarray(np.diagonal(alpha)).astype(np.float32)

    in_maps, perms = [], []
    for c in range(N_CORES):
        m = (s >> 5) == c
        orig_global = np.nonzero(m)[0]
        idx_dev, flat_pos = _route_queries(t[m], s[m] & (S - 1))

        o3 = obs[:, S * c:S * (c + 1)].reshape(J, P, S)
        obs1 = np.ascontiguousarray(o3.transpose(1, 0, 2)).reshape(P, J * S)
        par = np.ascontiguousarray(
            np.stack([mu[S * c:S * (c + 1)], adiag[S * c:S * (c + 1)]]))
        in_maps.append({
            "idx": idx_dev,
            "obs1": obs1.astype(np.int8),
            "par": par,
            "beta": beta,
        })
        perms.append((flat_pos, orig_global))
    return in_maps, perms


def kernel(t, s, obs, mu, alpha, beta, **_unused):
    nc = _get_nc()
    in_maps, perms = _make_in_maps(t, s, obs, mu, alpha, beta)
    res = run_bass_kernel_spmd(nc, in_maps, core_ids=list(range(N_CORES)))
    out = np.empty(BATCH, np.float32)
    for c in range(N_CORES):
        dev = res.results[c]["out"].reshape(-1)   # [P*NC]
        out[perms[c][1]] = dev[perms[c][0]]
    return out


if __name__ == "__main__":
    # quick self-check against a numpy re-implementation on random data
    rng = np.random.default_rng(0)
    t = rng.integers(0, N_TIME, BATCH).astype(np.int32)
    s = rng.integers(0, N_SPACE, BATCH).astype(np.int32)
    obs = rng.integers(0, 10, (N_TIME, N_SPACE)).astype(np.int32)
    mu = rng.random(N_SPACE, dtype=np.float32)
    alpha = rng.random((N_SPACE, N_SPACE), dtype=np.float32)
    beta = (rng.random(1, dtype=np.float32) + 0.1).astype(np.float32)

    got = kernel(t=t, s=s, obs=obs, mu=mu, alpha=alpha, beta=beta)

    b = float(beta[0])
    e = np.exp(-b)
    F = np.zeros((N_TIME, N_SPACE), np.float64)
    for tt in range(1, N_TIME):
        F[tt] = e * (F[tt - 1] + obs[tt - 1])
    G = np.clip(mu[None, :] + np.diag(alpha)[None, :] * b * F, LAM_MIN, None)
    want = G[t, s].astype(np.float32)
    err = np.abs(got - want) / np.maximum(np.abs(want), 1e-6)
    print("max rel err:", err.max(), "mean:", err.mean())


# revision 18
# speedup vs baseline: 3.4063x; 3.4063x over previous
"""Discrete Hawkes conditional-intensity kernel for 8 Trainium2 NeuronCores.

Math
----
Reference computes, per query i with (t, s) = (t_i, s_i):

    lam_i = clip(mu[s] + alpha[s, s] * b * F[t, s], 1e-5)
    F[t, s] = sum_{tp < t} obs[tp, s] * exp(-b * (t - tp))

With t = j*128 + p (j time-block of 128):

    F[j*128+p, s] = sum_{q<p} obs[j*128+q, s] e^{-b(p-q)}   (within block, PE)
                  + e^{-b p} * C[j, s]                       (carry)
    C[j, s] = F[j*128, s] = sum_{j'<j} e^{-128 b (j-1-j')} r[j', s]
    r[j, s] = sum_q obs[j*128+q, s] e^{-b(128-q)}

Sharding: by SPACE.  Core c owns s in [32c, 32c+32) — it reads only its
32 obs columns (1/8 of obs) and builds its G table [4096, 32] directly
in SBUF as G_sb[p, (j, s)] (one blocked matmul pass; r and the carry C
are two more small matmuls, not a sequential chain).  Queries
(~8192/core, those with s in the core's range) are answered by one
gpsimd ap_gather: query (t, s) lives on partition p = t mod 128 at
free offset u = (t div 128)*32 + s_rel; the host routes each query to
a slot in the 16-partition group containing p, and extracts
out[p, slot] from the dumped [128, NI] result.  G never leaves SBUF;
no collectives.
"""

import os
import sys

import numpy as np

_REPO_CANDIDATES = ("/opt/trn_rl_repo", os.path.expanduser("~/.axon_site/_ro/trn_rl_repo"))
for _p in _REPO_CANDIDATES:
    if os.path.isdir(_p) and _p not in sys.path:
        sys.path.append(_p)

import concourse.bass as bass
import concourse.tile as tile
from concourse import bacc, mybir
from concourse.bass_utils import run_bass_kernel_spmd

# Problem constants (hardcoded per spec).
N_TIME = 4096
N_SPACE = 256
BATCH = 65536
N_CORES = 8
LAM_MIN = 1e-5

P = 128                 # partitions / time-block size
J = N_TIME // P         # 32 time blocks
S = N_SPACE // N_CORES  # 32 space columns per core
NI = 1232               # gather slots per 16-partition group (max seen 1224)
GPC = P // 16           # 8 gpsimd cores / index groups

f32 = mybir.dt.float32
bf16 = mybir.dt.bfloat16
i32 = mybir.dt.int32
i16 = mybir.dt.int16
i8 = mybir.dt.int8
Alu = mybir.AluOpType
Act = mybir.ActivationFunctionType


def build_nc():
    nc = bacc.Bacc("TRN2", target_bir_lowering=False, debug=False)

    idx_h = nc.dram_tensor("idx", [P, NI // 16], i16, kind="ExternalInput")
    obs1_h = nc.dram_tensor("obs1", [P, J * S], i8, kind="ExternalInput")
    par_h = nc.dram_tensor("par", [2, S], f32, kind="ExternalInput")  # mu; adiag
    beta_h = nc.dram_tensor("beta", [1], f32, kind="ExternalInput")
    out_h = nc.dram_tensor("out", [P * NI], f32, kind="ExternalOutput")

    from contextlib import ExitStack

    with tile.TileContext(nc) as tc, ExitStack() as ctx:
        sb = ctx.enter_context(tc.tile_pool(name="sb", bufs=1))
        ps = ctx.enter_context(tc.tile_pool(name="ps", bufs=2, space="PSUM"))
        ps1 = ctx.enter_context(tc.tile_pool(name="ps1", bufs=2, space="PSUM"))

        # ---- input loads ------------------------------------------------
        beta_bc = sb.tile([P, 1], f32)
        nc.sync.dma_start(beta_bc[:], bass.AP(beta_h, 0, [[0, P], [1, 1]]))
        par = sb.tile([1, 2 * S], f32)   # [mu | adiag] on one partition
        nc.sync.dma_start(par[:], bass.AP(par_h, 0, [[1, 1], [1, 2 * S]]))
        obs1_i = sb.tile([P, J * S], i8)
        nc.sync.dma_start(obs1_i[:], obs1_h.ap())
        idx = sb.tile([P, NI // 16], i16)
        nc.scalar.dma_start(idx[:], idx_h.ap())

        # ---- runtime constants from beta --------------------------------
        negb = sb.tile([P, 1], f32)
        nc.vector.tensor_scalar(out=negb[:], in0=beta_bc[:], scalar1=-1.0,
                                scalar2=None, op0=Alu.mult)
        negb128 = sb.tile([J, 1], f32)
        nc.vector.tensor_scalar(out=negb128[:], in0=beta_bc[:J, :], scalar1=-128.0,
                                scalar2=None, op0=Alu.mult)

        # broadcast adiag to all 128 partitions via PE; asbb[s] = b*alpha[s,s]
        ones1 = sb.tile([1, P], f32)
        nc.vector.memset(ones1[:], 1.0)
        bc_ps = ps1.tile([P, S], f32)
        nc.tensor.matmul(bc_ps[:], lhsT=ones1[:], rhs=par[:, S:2 * S],
                         start=True, stop=True)
        asbb_bc = sb.tile([P, S], f32)
        nc.vector.tensor_scalar(out=asbb_bc[:], in0=bc_ps[:],
                                scalar1=beta_bc[:], scalar2=None, op0=Alu.mult)

        # v column: exp(b*(p-128))  (end-of-block carry weights)
        xvc = sb.tile([P, 1], i32)
        nc.gpsimd.iota(xvc[:], [[0, 1]], base=-P, channel_multiplier=1)
        vmc = sb.tile([P, 1], f32)
        nc.vector.tensor_scalar(out=vmc[:], in0=xvc[:], scalar1=beta_bc[:],
                                scalar2=None, op0=Alu.mult)
        vcolb = sb.tile([P, 1], bf16)
        nc.scalar.activation(vcolb[:], vmc[:], Act.Exp)

        # u2b: row0 = exp(-b p) (carry decay), row1 = ones (mu term)
        xu = sb.tile([1, P], i32)
        nc.gpsimd.iota(xu[:], [[1, P]], base=0, channel_multiplier=0)
        u2b = sb.tile([2, P], bf16)
        nc.vector.memset(u2b[:], 1.0)
        um = sb.tile([1, P], f32)
        nc.vector.tensor_scalar(out=um[:], in0=xu[:], scalar1=negb[:1, :],
                                scalar2=None, op0=Alu.mult)
        nc.scalar.activation(u2b[0:1, :], um[:], Act.Exp)

        # LdT[q, p] = exp(-b (p - q)) for q < p else 0   (within-block decay)
        xd = sb.tile([P, P], i32)
        nc.gpsimd.iota(xd[:], [[1, P]], base=0, channel_multiplier=-1)   # f - p
        lda = sb.tile([P, P], f32)
        nc.vector.tensor_scalar(out=lda[:], in0=xd[:], scalar1=negb[:],
                                scalar2=None, op0=Alu.mult)
        ldm = sb.tile([P, P], f32)
        nc.gpsimd.affine_select(ldm[:], lda[:], [[1, P]], Alu.is_gt, -90.0,
                                base=0, channel_multiplier=-1)
        ldtb = sb.tile([P, P], bf16)
        nc.scalar.activation(ldtb[:], ldm[:], Act.Exp)

        # K[j', j] = exp(-128 b (j - 1 - j')) for j' <= j-1 else 0  (carry)
        xc = sb.tile([J, J], i32)
        nc.gpsimd.iota(xc[:], [[1, J]], base=-1, channel_multiplier=-1)  # f - 1 - p
        lca = sb.tile([J, J], f32)
        nc.vector.tensor_scalar(out=lca[:], in0=xc[:], scalar1=negb128[:],
                                scalar2=None, op0=Alu.mult)
        lcm = sb.tile([J, J], f32)
        nc.gpsimd.affine_select(lcm[:], lca[:], [[1, J]], Alu.is_ge, -90.0,
                                base=-1, channel_multiplier=-1)
        kct = sb.tile([J, J], f32)
        nc.scalar.activation(kct[:], lcm[:], Act.Exp)

        # ---- obs scale + carry path (all matmuls) -----------------------
        # obs_f1[p, (j, s)] = obs1 * asbb[s]
        obs_f1 = sb.tile([P, J * S], bf16)
        nc.vector.tensor_tensor(
            out=obs_f1[:].rearrange("p (j s) -> p j s", s=S),
            in0=obs1_i[:].rearrange("p (j s) -> p j s", s=S),
            in1=asbb_bc[:].unsqueeze(1).broadcast_to((P, J, S)),
            op=Alu.mult)

        HALF = J * S // 2   # 512 free elems per PSUM bank
        # r[(j, s)] = sum_q obs_f1[q, (j, s)] * v[q]
        r_flat = sb.tile([1, J * S], f32)
        for h in range(2):
            r_ps = ps.tile([1, HALF], f32)
            nc.tensor.matmul(r_ps[:], lhsT=vcolb[:],
                             rhs=obs_f1[:, h * HALF:(h + 1) * HALF],
                             start=True, stop=True)
            if h == 0:
                nc.scalar.activation(r_flat[:, 0:HALF], r_ps[:], Act.Copy)
            else:
                nc.vector.tensor_copy(r_flat[:, HALF:2 * HALF], r_ps[:])
        r32 = sb.tile([J, S], f32)
        nc.sync.dma_start(r32[:], r_flat[:])
        c_ps = ps1.tile([J, S], f32)
        nc.tensor.matmul(c_ps[:], lhsT=kct[:], rhs=r32[:], start=True, stop=True)
        c_sb = sb.tile([J, S], bf16)
        nc.vector.tensor_copy(c_sb[:], c_ps[:])

        # rhs2: row0 = C flat (sbuf reshape dma), row1 = mu tiled (dma bcast)
        rhs2 = sb.tile([2, J * S], bf16)
        mu_b = sb.tile([1, S], bf16)
        nc.vector.tensor_copy(mu_b[:], par[:, 0:S])
        nc.scalar.dma_start(
            rhs2[1:2, :].rearrange("o (j s) -> o j s", s=S),
            mu_b[:].unsqueeze(1).broadcast_to((1, J, S)))
        nc.sync.dma_start(
            rhs2[0:1, :].rearrange("o (j s) -> o j s", s=S), c_sb[:])

        # ---- G build (SBUF only), clip fused into the PSUM->SBUF copy ---
        g_sb = sb.tile([P, J * S], f32)
        for h in range(2):
            pch = ps.tile([P, HALF], f32)
            nc.tensor.matmul(pch[:], lhsT=ldtb[:],
                             rhs=obs_f1[:, h * HALF:(h + 1) * HALF],
                             start=True, stop=True)
            nc.tensor.matmul(pch[:], lhsT=u2b[:],
                             rhs=rhs2[:, h * HALF:(h + 1) * HALF],
                             start=False, stop=True, skip_group_check=True)
            nc.vector.tensor_scalar(
                out=g_sb[:, h * HALF:(h + 1) * HALF], in0=pch[:],
                scalar1=float(LAM_MIN), scalar2=None, op0=Alu.max)

        # ---- gather + out ----------------------------------------------
        gout = sb.tile([P, NI], f32)
        nc.gpsimd.ap_gather(
            out_ap=gout[:], in_ap=g_sb[:], idxs_ap=idx[:],
            channels=P, num_elems=J * S, d=1, num_idxs=NI)
        nc.sync.dma_start(bass.AP(out_h, 0, [[NI, P], [1, NI]]), gout[:])

    nc.compile()
    return nc


_NC_CACHE = None


def _get_nc():
    global _NC_CACHE
    if _NC_CACHE is None:
        _NC_CACHE = build_nc()
    return _NC_CACHE


def _route_queries(tc_, sc_):
    """Route a core's queries to ap_gather slots.

    Query (t, s) lives on partition p = t mod 128, which belongs to
    16-partition group g = p >> 4; its table offset is u = (t >> 7)*S + s.
    Group g's index list (NI entries, wrapped (slot % 16, slot // 16) over
    partitions [16g, 16g+16)) holds u at the query's slot; the result is
    read from out[p, slot].  Returns (idx_dev [P, NI//16] int16, flat
    positions p*NI+slot per query in input order)."""
    n = tc_.shape[0]
    p = tc_ % P
    g = p >> 4
    u = ((tc_ >> 7) * S + sc_).astype(np.int16)
    order = np.argsort(g, kind="stable")
    counts = np.bincount(g, minlength=GPC)
    if counts.max() > NI:
        raise RuntimeError("group query count exceeds NI slots")
    slot = np.empty(n, np.int64)
    starts = np.zeros(GPC, np.int64)
    np.cumsum(counts[:-1], out=starts[1:])
    slot[order] = np.arange(n) - starts[g[order]]
    idx_dev = np.zeros((P, NI // 16), np.int16)
    idx_dev[(g << 4) + (slot % 16).astype(np.int64), slot >> 4] = u
    return idx_dev, p.astype(np.int64) * NI + slot


def _make_in_maps(t, s, obs, mu, alpha, beta):
    """Shard by space: core c gets s in [S*c, S*(c+1)).  Returns
    (in_maps, perms) where perms[c] = (flat_out_pos, global_orig_pos)."""
    t = np.ascontiguousarray(np.asarray(t, dtype=np.int32))
    s = np.ascontiguousarray(np.asarray(s, dtype=np.int32))
    obs = np.ascontiguousarray(np.asarray(obs, dtype=np.int32))
    mu = np.ascontiguousarray(np.asarray(mu, dtype=np.float32))
    alpha = np.asarray(alpha, dtype=np.float32)
    beta = np.ascontiguousarray(np.asarray(beta, dtype=np.float32))
    adiag = np.ascontiguousarray(np.diagonal(alpha)).astype(np.float32)

    in_maps, perms = [], []
    for c in range(N_CORES):
        m = (s >> 5) == c
        orig_global = np.nonzero(m)[0]
        idx_dev, flat_pos = _route_queries(t[m], s[m] & (S - 1))

        o3 = obs[:, S * c:S * (c + 1)].reshape(J, P, S)
        obs1 = np.ascontiguousarray(o3.transpose(1, 0, 2)).reshape(P, J * S)
        par = np.ascontiguousarray(
            np.stack([mu[S * c:S * (c + 1)], adiag[S * c:S * (c + 1)]]))
        in_maps.append({
            "idx": idx_dev,
            "obs1": obs1.astype(np.int8),
            "par": par,
            "beta": beta,
        })
        perms.append((flat_pos, orig_global))
    return in_maps, perms


def kernel(t, s, obs, mu, alpha, beta, **_unused):
    nc = _get_nc()
    in_maps, perms = _make_in_maps(t, s, obs, mu, alpha, beta)
    res = run_bass_kernel_spmd(nc, in_maps, core_ids=list(range(N_CORES)))
    out = np.empty(BATCH, np.float32)
    for c in range(N_CORES):
        dev = res.results[c]["out"].reshape(-1)   # [P*NI]
        out[perms[c][1]] = dev[perms[c][0]]
    return out


if __name__ == "__main__":
    # quick self-check against a numpy re-implementation on random data
    rng = np.random.default_rng(0)
    t = rng.integers(0, N_TIME, BATCH).astype(np.int32)
    s = rng.integers(0, N_SPACE, BATCH).astype(np.int32)
    obs = rng.integers(0, 10, (N_TIME, N_SPACE)).astype(np.int32)
    mu = rng.random(N_SPACE, dtype=np.float32)
    alpha = rng.random((N_SPACE, N_SPACE), dtype=np.float32)
    beta = (rng.random(1, dtype=np.float32) + 0.1).astype(np.float32)

    got = kernel(t=t, s=s, obs=obs, mu=mu, alpha=alpha, beta=beta)

    b = float(beta[0])
    e = np.exp(-b)
    F = np.zeros((N_TIME, N_SPACE), np.float64)
    for tt in range(1, N_TIME):
        F[tt] = e * (F[tt - 1] + obs[tt - 1])
    G = np.clip(mu[None, :] + np.diag(alpha)[None, :] * b * F, LAM_MIN, None)
    want = G[t, s].astype(np.float32)
    err = np.abs(got - want) / np.maximum(np.abs(want), 1e-6)
    print("max rel err:", err.max(), "mean:", err.mean())


# revision 19
# speedup vs baseline: 8.2401x; 2.4191x over previous
"""Discrete Hawkes conditional-intensity kernel for 8 Trainium2 NeuronCores.

Math
----
Reference computes, per query i with (t, s) = (t_i, s_i):

    lam_i = clip(mu[s] + alpha[s, s] * b * F[t, s], 1e-5)
    F[t, s] = sum_{tp < t} obs[tp, s] * exp(-b * (t - tp))

With t = j*128 + p (j time-block of 128):

    F[j*128+p, s] = sum_{q<p} obs[j*128+q, s] e^{-b(p-q)}   (within block, PE)
                  + e^{-b p} * C[j, s]                       (carry)
    C[j, s] = F[j*128, s] = sum_{j'<j} e^{-128 b (j-1-j')} r[j', s]
    r[j, s] = sum_q obs[j*128+q, s] e^{-b(128-q)}

Sharding: by SPACE.  Core c owns s in [32c, 32c+32) — it reads only its
32 obs columns (1/8 of obs) and builds its G table [4096, 32] directly
in SBUF as G_sb[p, (j, s)] (one blocked matmul pass; r and the carry C
are two more small matmuls, not a sequential chain).  The whole
table (one 512KB slice per core, jointly the full 4MB G) is dumped to
DRAM and the host picks each query's cell out[p, u] from its core's
slice (p = t mod 128, u = (t div 128)*32 + s_rel) while un-sharding —
the same index-permutation step the output path needs anyway.  No
gather instructions, no collectives.
"""

import os
import sys

import numpy as np

_REPO_CANDIDATES = ("/opt/trn_rl_repo", os.path.expanduser("~/.axon_site/_ro/trn_rl_repo"))
for _p in _REPO_CANDIDATES:
    if os.path.isdir(_p) and _p not in sys.path:
        sys.path.append(_p)

import concourse.bass as bass
import concourse.tile as tile
from concourse import bacc, mybir
from concourse.bass_utils import run_bass_kernel_spmd

# Problem constants (hardcoded per spec).
N_TIME = 4096
N_SPACE = 256
BATCH = 65536
N_CORES = 8
LAM_MIN = 1e-5

P = 128                 # partitions / time-block size
J = N_TIME // P         # 32 time blocks
S = N_SPACE // N_CORES  # 32 space columns per core

f32 = mybir.dt.float32
bf16 = mybir.dt.bfloat16
i32 = mybir.dt.int32
i16 = mybir.dt.int16
i8 = mybir.dt.int8
Alu = mybir.AluOpType
Act = mybir.ActivationFunctionType


def build_nc():
    nc = bacc.Bacc("TRN2", target_bir_lowering=False, debug=False)

    obs1_h = nc.dram_tensor("obs1", [P, J * S], i8, kind="ExternalInput")
    par_h = nc.dram_tensor("par", [2, S], f32, kind="ExternalInput")  # mu; adiag
    beta_h = nc.dram_tensor("beta", [1], f32, kind="ExternalInput")
    out_h = nc.dram_tensor("out", [P * J * S], f32, kind="ExternalOutput")

    from contextlib import ExitStack

    with tile.TileContext(nc) as tc, ExitStack() as ctx:
        sb = ctx.enter_context(tc.tile_pool(name="sb", bufs=1))
        ps = ctx.enter_context(tc.tile_pool(name="ps", bufs=2, space="PSUM"))
        ps1 = ctx.enter_context(tc.tile_pool(name="ps1", bufs=2, space="PSUM"))

        # ---- input loads ------------------------------------------------
        beta_bc = sb.tile([P, 1], f32)
        nc.sync.dma_start(beta_bc[:], bass.AP(beta_h, 0, [[0, P], [1, 1]]))
        par = sb.tile([1, 2 * S], f32)   # [mu | adiag] on one partition
        nc.sync.dma_start(par[:], bass.AP(par_h, 0, [[1, 1], [1, 2 * S]]))
        obs1_i = sb.tile([P, J * S], i8)
        nc.sync.dma_start(obs1_i[:], obs1_h.ap())

        # ---- runtime constants from beta --------------------------------
        negb = sb.tile([P, 1], f32)
        nc.vector.tensor_scalar(out=negb[:], in0=beta_bc[:], scalar1=-1.0,
                                scalar2=None, op0=Alu.mult)
        # broadcast adiag to all 128 partitions via PE; asbb[s] = b*alpha[s,s]
        ones1 = sb.tile([1, P], f32)
        nc.vector.memset(ones1[:], 1.0)
        bc_ps = ps1.tile([P, S], f32)
        nc.tensor.matmul(bc_ps[:], lhsT=ones1[:], rhs=par[:, S:2 * S],
                         start=True, stop=True)
        asbb_bc = sb.tile([P, S], f32)
        nc.vector.tensor_scalar(out=asbb_bc[:], in0=bc_ps[:],
                                scalar1=beta_bc[:], scalar2=None, op0=Alu.mult)

        # obs_f1[p, (j, s)] = obs1 * asbb[s]
        obs_f1 = sb.tile([P, J * S], bf16)
        nc.vector.tensor_tensor(
            out=obs_f1[:].rearrange("p (j s) -> p j s", s=S),
            in0=obs1_i[:].rearrange("p (j s) -> p j s", s=S),
            in1=asbb_bc[:].unsqueeze(1).broadcast_to((P, J, S)),
            op=Alu.mult)

        negb128 = sb.tile([J, 1], f32)
        nc.vector.tensor_scalar(out=negb128[:], in0=beta_bc[:J, :], scalar1=-128.0,
                                scalar2=None, op0=Alu.mult)

        # v column: exp(b*(p-128))  (end-of-block carry weights)
        xvc = sb.tile([P, 1], i32)
        nc.gpsimd.iota(xvc[:], [[0, 1]], base=-P, channel_multiplier=1)
        vmc = sb.tile([P, 1], f32)
        nc.vector.tensor_scalar(out=vmc[:], in0=xvc[:], scalar1=beta_bc[:],
                                scalar2=None, op0=Alu.mult)
        vcolb = sb.tile([P, 1], bf16)
        nc.scalar.activation(vcolb[:], vmc[:], Act.Exp)

        # u2b: row0 = exp(-b p) (carry decay), row1 = ones (mu term)
        xu = sb.tile([1, P], i32)
        nc.gpsimd.iota(xu[:], [[1, P]], base=0, channel_multiplier=0)
        u2b = sb.tile([2, P], bf16)
        nc.vector.memset(u2b[:], 1.0)
        um = sb.tile([1, P], f32)
        nc.vector.tensor_scalar(out=um[:], in0=xu[:], scalar1=negb[:1, :],
                                scalar2=None, op0=Alu.mult)
        nc.scalar.activation(u2b[0:1, :], um[:], Act.Exp)

        # LdT[q, p] = exp(-b (p - q)) for q < p else 0   (within-block decay)
        xd = sb.tile([P, P], i32)
        nc.gpsimd.iota(xd[:], [[1, P]], base=0, channel_multiplier=-1)   # f - p
        lda = sb.tile([P, P], f32)
        nc.vector.tensor_scalar(out=lda[:], in0=xd[:], scalar1=negb[:],
                                scalar2=None, op0=Alu.mult)
        ldm = sb.tile([P, P], f32)
        nc.gpsimd.affine_select(ldm[:], lda[:], [[1, P]], Alu.is_gt, -90.0,
                                base=0, channel_multiplier=-1)
        ldtb = sb.tile([P, P], bf16)
        nc.scalar.activation(ldtb[:], ldm[:], Act.Exp)

        # K[j', j] = exp(-128 b (j - 1 - j')) for j' <= j-1 else 0  (carry)
        xc = sb.tile([J, J], i32)
        nc.gpsimd.iota(xc[:], [[1, J]], base=-1, channel_multiplier=-1)  # f - 1 - p
        lca = sb.tile([J, J], f32)
        nc.vector.tensor_scalar(out=lca[:], in0=xc[:], scalar1=negb128[:],
                                scalar2=None, op0=Alu.mult)
        lcm = sb.tile([J, J], f32)
        nc.gpsimd.affine_select(lcm[:], lca[:], [[1, J]], Alu.is_ge, -90.0,
                                base=-1, channel_multiplier=-1)
        kct = sb.tile([J, J], f32)
        nc.scalar.activation(kct[:], lcm[:], Act.Exp)

        # ---- carry path (all matmuls) -----------------------------------
        HALF = J * S // 2   # 512 free elems per PSUM bank
        # r[(j, s)] = sum_q obs_f1[q, (j, s)] * v[q]
        r_flat = sb.tile([1, J * S], f32)
        for h in range(2):
            r_ps = ps.tile([1, HALF], f32)
            nc.tensor.matmul(r_ps[:], lhsT=vcolb[:],
                             rhs=obs_f1[:, h * HALF:(h + 1) * HALF],
                             start=True, stop=True)
            if h == 0:
                nc.scalar.activation(r_flat[:, 0:HALF], r_ps[:], Act.Copy)
            else:
                nc.vector.tensor_copy(r_flat[:, HALF:2 * HALF], r_ps[:])
        r32 = sb.tile([J, S], f32)
        nc.sync.dma_start(r32[:], r_flat[:])
        c_ps = ps1.tile([J, S], f32)
        nc.tensor.matmul(c_ps[:], lhsT=kct[:], rhs=r32[:], start=True, stop=True)
        c_sb = sb.tile([J, S], bf16)
        nc.vector.tensor_copy(c_sb[:], c_ps[:])

        # rhs2: row0 = C flat (sbuf reshape dma), row1 = mu tiled (dma bcast)
        rhs2 = sb.tile([2, J * S], bf16)
        mu_b = sb.tile([1, S], bf16)
        nc.vector.tensor_copy(mu_b[:], par[:, 0:S])
        nc.scalar.dma_start(
            rhs2[1:2, :].rearrange("o (j s) -> o j s", s=S),
            mu_b[:].unsqueeze(1).broadcast_to((1, J, S)))
        nc.sync.dma_start(
            rhs2[0:1, :].rearrange("o (j s) -> o j s", s=S), c_sb[:])

        # ---- G build, clip fused into the PSUM->SBUF copy, dump ---------
        g_sb = sb.tile([P, J * S], f32)
        for h in range(2):
            pch = ps.tile([P, HALF], f32)
            nc.tensor.matmul(pch[:], lhsT=ldtb[:],
                             rhs=obs_f1[:, h * HALF:(h + 1) * HALF],
                             start=True, stop=True)
            nc.tensor.matmul(pch[:], lhsT=u2b[:],
                             rhs=rhs2[:, h * HALF:(h + 1) * HALF],
                             start=False, stop=True, skip_group_check=True)
            nc.vector.tensor_scalar(
                out=g_sb[:, h * HALF:(h + 1) * HALF], in0=pch[:],
                scalar1=float(LAM_MIN), scalar2=None, op0=Alu.max)
            eng = nc.sync if h == 0 else nc.scalar
            eng.dma_start(
                bass.AP(out_h, h * HALF, [[J * S, P], [1, HALF]]),
                g_sb[:, h * HALF:(h + 1) * HALF])

    nc.compile()
    return nc


_NC_CACHE = None


def _get_nc():
    global _NC_CACHE
    if _NC_CACHE is None:
        _NC_CACHE = build_nc()
    return _NC_CACHE


def _flat_positions(tc_, sc_):
    """Query (t, s) is table cell [p = t mod 128, u = (t div 128)*S + s]
    of the dumped [128, J*S] slice."""
    return (tc_ % P).astype(np.int64) * (J * S) + (tc_ >> 7) * S + sc_


def _make_in_maps(t, s, obs, mu, alpha, beta):
    """Shard by space: core c gets s in [S*c, S*(c+1)).  Returns
    (in_maps, perms) where perms[c] = (flat_out_pos, global_orig_pos)."""
    t = np.ascontiguousarray(np.asarray(t, dtype=np.int32))
    s = np.ascontiguousarray(np.asarray(s, dtype=np.int32))
    obs = np.ascontiguousarray(np.asarray(obs, dtype=np.int32))
    mu = np.ascontiguousarray(np.asarray(mu, dtype=np.float32))
    alpha = np.asarray(alpha, dtype=np.float32)
    beta = np.ascontiguousarray(np.asarray(beta, dtype=np.float32))
    adiag = np.ascontiguousarray(np.diagonal(alpha)).astype(np.float32)

    in_maps, perms = [], []
    for c in range(N_CORES):
        m = (s >> 5) == c
        orig_global = np.nonzero(m)[0]
        flat_pos = _flat_positions(t[m], s[m] & (S - 1))

        o3 = obs[:, S * c:S * (c + 1)].reshape(J, P, S)
        obs1 = np.ascontiguousarray(o3.transpose(1, 0, 2)).reshape(P, J * S)
        par = np.ascontiguousarray(
            np.stack([mu[S * c:S * (c + 1)], adiag[S * c:S * (c + 1)]]))
        in_maps.append({
            "obs1": obs1.astype(np.int8),
            "par": par,
            "beta": beta,
        })
        perms.append((flat_pos, orig_global))
    return in_maps, perms


def kernel(t, s, obs, mu, alpha, beta, **_unused):
    nc = _get_nc()
    in_maps, perms = _make_in_maps(t, s, obs, mu, alpha, beta)
    res = run_bass_kernel_spmd(nc, in_maps, core_ids=list(range(N_CORES)))
    out = np.empty(BATCH, np.float32)
    for c in range(N_CORES):
        dev = res.results[c]["out"].reshape(-1)   # [P*J*S]
        out[perms[c][1]] = dev[perms[c][0]]
    return out


if __name__ == "__main__":
    # quick self-check against a numpy re-implementation on random data
    rng = np.random.default_rng(0)
    t = rng.integers(0, N_TIME, BATCH).astype(np.int32)
    s = rng.integers(0, N_SPACE, BATCH).astype(np.int32)
    obs = rng.integers(0, 10, (N_TIME, N_SPACE)).astype(np.int32)
    mu = rng.random(N_SPACE, dtype=np.float32)
    alpha = rng.random((N_SPACE, N_SPACE), dtype=np.float32)
    beta = (rng.random(1, dtype=np.float32) + 0.1).astype(np.float32)

    got = kernel(t=t, s=s, obs=obs, mu=mu, alpha=alpha, beta=beta)

    b = float(beta[0])
    e = np.exp(-b)
    F = np.zeros((N_TIME, N_SPACE), np.float64)
    for tt in range(1, N_TIME):
        F[tt] = e * (F[tt - 1] + obs[tt - 1])
    G = np.clip(mu[None, :] + np.diag(alpha)[None, :] * b * F, LAM_MIN, None)
    want = G[t, s].astype(np.float32)
    err = np.abs(got - want) / np.maximum(np.abs(want), 1e-6)
    print("max rel err:", err.max(), "mean:", err.mean())
